# revision 11
# baseline (speedup 1.0000x reference)
"""Trainium2 Bass kernel for BNN VGG7 (nn_BNN_VGG7_32461362823713).

Strategy
--------
Pure data parallelism: batch 128 -> 16 images per NeuronCore x 8 cores.
All weights / BN params are host-folded and replicated.

Numerics: after block1, every activation is +-1 and all conv/fc arithmetic
is exact integer arithmetic in fp32 PSUM (bf16 operands are exactly +-1,
products exact, partial sums < 2^24). BatchNorm+Hardtanh+binarize folds to
sign(x - t) per channel with t = m - b/(g*rsqrt(v+eps)) (g>0 checked).
Conv1 sees the real-valued input x: it is computed EXACTLY by splitting
x = xh + xl where xh is on a 2^-16 grid (3 bf16 components; all partial
sums are grid multiples < 2^24 -> fp32 accumulation is exact) and xl is
the residual (2 bf16 components, |xl|<=2^-17, accumulation error ~1e-9).
sign((S_lo - t) + S_hi) then matches the fp32 reference bit-for-bit
(validated: min |conv1 - threshold| gap for this network is ~8.5e-7,
our error < 3e-7).
"""

import os
import sys

import numpy as np

if "/opt/trn_rl_repo" not in sys.path:
    sys.path.insert(0, "/opt/trn_rl_repo")

import ml_dtypes

import concourse.bass as bass
import concourse.mybir as mybir
import concourse.tile as tile
from concourse import bacc
from concourse.masks import make_identity

BF16 = mybir.dt.bfloat16
F32 = mybir.dt.float32
NP_BF16 = ml_dtypes.bfloat16

N_CORES = 8
B_FULL = 128
B = B_FULL // N_CORES  # 16 images per core
EPS = 1e-5

# ---------------------------------------------------------------------------
# Host-side preprocessing
# ---------------------------------------------------------------------------


def _binarize(w):
    return np.where(w >= 0, np.float32(1.0), np.float32(-1.0))


def _fold_threshold(g, b, m, v):
    inv = g / np.sqrt(v + np.float32(EPS), dtype=np.float32)
    assert (inv > 0).all(), "negative BN scale not supported by sign-folding"
    return (m - b / inv).astype(np.float32)


def _split_x(x):
    """Exact split of fp32 x into 3 hi bf16 parts (2^-16 grid) + 2 lo parts."""
    g = np.float32(2.0**-16)
    xh = (np.round(x / g).astype(np.float64) * np.float64(g)).astype(np.float32)
    xl = (x.astype(np.float64) - xh.astype(np.float64)).astype(np.float32)

    def bf_split(a, n):
        parts = []
        r = a.copy()
        for _ in range(n):
            p = r.astype(NP_BF16)
            parts.append(p)
            r = (r - p.astype(np.float32)).astype(np.float32)
        return parts

    return bf_split(xh, 3), bf_split(xl, 2)


def _im2row(parts, w_shape=32):
    """parts: list of [B,3,H,W] bf16 -> [len(parts)*27, B, H, W] bf16.

    Partition index p = tap*9 + part*3 + ch, tap = dy*3+dx, taken from the
    replicate-padded image so each tap row is the shifted full-size window.
    """
    npart = len(parts)
    Bn, C, H, W = parts[0].shape
    out = np.empty((9 * npart * 3, Bn, H, W), dtype=NP_BF16)
    for pi, p in enumerate(parts):
        pp = np.pad(p, ((0, 0), (0, 0), (1, 1), (1, 1)), mode="edge")
        for tap in range(9):
            dy, dx = tap // 3, tap % 3
            blk = pp[:, :, dy : dy + H, dx : dx + W]  # [B,3,H,W]
            for ch in range(3):
                out[tap * 3 * npart + pi * 3 + ch] = blk[:, ch]
    return out


def _conv_w_dev(wb, n_kc, n_mc):
    """wb [O, I, 3, 3] +-1 fp32 -> [128, n_kc, n_mc, 9, 128] bf16 lhsT table."""
    O, I = wb.shape[0], wb.shape[1]
    assert I == n_kc * 128 and O == n_mc * 128
    out = np.empty((128, n_kc, n_mc, 9, 128), dtype=NP_BF16)
    for kc in range(n_kc):
        for mc in range(n_mc):
            for tap in range(9):
                dy, dx = tap // 3, tap % 3
                # lhsT[ic_local, oc_local]
                out[:, kc, mc, tap, :] = (
                    wb[mc * 128 : (mc + 1) * 128, kc * 128 : (kc + 1) * 128, dy, dx]
                    .T.astype(NP_BF16)
                )
    return out


def _prep(inputs):
    """Host preprocessing -> (shared tensors dict, per-core in_maps list)."""
    x = np.asarray(inputs["x"], dtype=np.float32)

    t = {i: _fold_threshold(*(np.asarray(inputs[f"{k}{i}"], np.float32) for k in "gbmv"))
         for i in range(1, 8)}
    wb = {i: _binarize(np.asarray(inputs[f"w{i}"], np.float32)) for i in range(1, 7)}
    fc1b = _binarize(np.asarray(inputs["fc1_w"], np.float32))
    fc2b = _binarize(np.asarray(inputs["fc2_w"], np.float32))
    scale = np.asarray(inputs["scale"], np.float32).reshape(1)

    # conv1 lhsT tables: [81,128] / [54,128], row p = tap*9+part*3+ch
    def c1_lhsT(nparts):
        out = np.empty((9 * nparts * 3, 128), dtype=NP_BF16)
        for pi in range(nparts):
            for tap in range(9):
                dy, dx = tap // 3, tap % 3
                for ch in range(3):
                    out[tap * 3 * nparts + pi * 3 + ch] = (
                        wb[1][:, ch, dy, dx].astype(NP_BF16))
        return out

    shared = {
        "w1hi": c1_lhsT(3),
        "w1lo": c1_lhsT(2),
        "w2": _conv_w_dev(wb[2], 1, 1).reshape(128, -1),
        "w3": _conv_w_dev(wb[3], 1, 2).reshape(128, -1),
        "w4": _conv_w_dev(wb[4], 2, 2).reshape(128, -1),
        "w5": _conv_w_dev(wb[5], 2, 4).reshape(128, -1),
        "w6": _conv_w_dev(wb[6], 4, 4).reshape(128, -1),
        # negated thresholds (ACT computes sign(in + bias))
        "nt1": (-t[1]).reshape(128, 1).copy(),
        "nt2": (-t[2]).reshape(128, 1).copy(),
        "nt3": (-t[3]).reshape(2, 128).T.copy(),   # [128, mc]
        "nt4": (-t[4]).reshape(2, 128).T.copy(),
        "nt5": (-t[5]).reshape(4, 128).T.copy(),
        "nt6": (-t[6]).reshape(4, 128).T.copy(),
        "nt7": (-t[7]).reshape(8, 128).T.copy(),   # [128, chunk]
        "scale_in": np.repeat(scale, 10).reshape(10, 1).copy(),
    }

    # fc1 weights: step s = cg*16 + pix -> [128 c_local, 1024 oc]
    # fc1_flat k = c*16 + pix with c global over 512.
    w = fc1b.astype(NP_BF16)  # [1024, 8192]
    fc1_dev = np.empty((64, 128, 1024), dtype=NP_BF16)
    for cg in range(4):
        for pix in range(16):
            ks = (128 * cg + np.arange(128)) * 16 + pix
            fc1_dev[cg * 16 + pix] = w[:, ks].T
    shared["fc1"] = fc1_dev

    # fc2: [128, 8, 10] bf16, chunk j rows k=128j..128j+127
    fc2_dev = np.empty((128, 8, 10), dtype=NP_BF16)
    for j in range(8):
        fc2_dev[:, j, :] = fc2b[:, 128 * j : 128 * (j + 1)].T.astype(NP_BF16)
    shared["fc2"] = fc2_dev

    hi_parts, lo_parts = _split_x(x)
    hi_all = _im2row(hi_parts)  # [81, 128, 32, 32]
    lo_all = _im2row(lo_parts)  # [54, 128, 32, 32]

    in_maps = []
    for c in range(N_CORES):
        m = dict(shared)
        m["hi"] = np.ascontiguousarray(
            hi_all[:, c * B : (c + 1) * B].reshape(81, B * 1024))
        m["lo"] = np.ascontiguousarray(
            lo_all[:, c * B : (c + 1) * B].reshape(54, B * 1024))
        in_maps.append(m)
    return in_maps


# ---------------------------------------------------------------------------
# Bass program
# ---------------------------------------------------------------------------


def _dma_in(nc, tile_ap, dram_ap, splits=4):
    """Load dram->sbuf split along the free dim for DMA queue parallelism."""
    total = tile_ap.shape[-1]
    if splits <= 1 or total % splits != 0 or len(tile_ap.shape) != 2:
        nc.sync.dma_start(out=tile_ap, in_=dram_ap)
        return
    step = total // splits
    for i in range(splits):
        nc.sync.dma_start(
            out=tile_ap[:, i * step : (i + 1) * step],
            in_=dram_ap[:, i * step : (i + 1) * step],
        )


def build_program():
    nc = bacc.Bacc("TRN2", target_bir_lowering=False, debug=False, num_devices=N_CORES)

    d = {}
    d["hi"] = nc.dram_tensor("hi", [81, B * 1024], BF16, kind="ExternalInput")
    d["lo"] = nc.dram_tensor("lo", [54, B * 1024], BF16, kind="ExternalInput")
    d["w1hi"] = nc.dram_tensor("w1hi", [81, 128], BF16, kind="ExternalInput")
    d["w1lo"] = nc.dram_tensor("w1lo", [54, 128], BF16, kind="ExternalInput")
    wcols = {2: 1152, 3: 2304, 4: 4608, 5: 9216, 6: 18432}
    for i in range(2, 7):
        d[f"w{i}"] = nc.dram_tensor(f"w{i}", [128, wcols[i]], BF16, kind="ExternalInput")
    ntc = {1: 1, 2: 1, 3: 2, 4: 2, 5: 4, 6: 4, 7: 8}
    for i in range(1, 8):
        d[f"nt{i}"] = nc.dram_tensor(f"nt{i}", [128, ntc[i]], F32, kind="ExternalInput")
    d["fc1"] = nc.dram_tensor("fc1", [64, 128, 1024], BF16, kind="ExternalInput")
    d["fc2"] = nc.dram_tensor("fc2", [128, 8, 10], BF16, kind="ExternalInput")
    d["scale_in"] = nc.dram_tensor("scale_in", [10, 1], F32, kind="ExternalInput")
    d["out"] = nc.dram_tensor("out", [B, 10], F32, kind="ExternalOutput")

    with tile.TileContext(nc) as tc:
        _emit(nc, tc, d)
    nc.compile()
    return nc


def _emit(nc, tc, d):
    from contextlib import ExitStack

    AF = mybir.ActivationFunctionType

    est = ExitStack()
    with est:
        wpool = est.enter_context(tc.tile_pool(name="wpool", bufs=1))
        apool = est.enter_context(tc.tile_pool(name="apool", bufs=1))
        psum = est.enter_context(tc.tile_pool(name="psum", bufs=4, space="PSUM"))
        tmp = est.enter_context(tc.tile_pool(name="tmp", bufs=4))
        # psum: tag "cps" gets the pool default 4 banks; fc-era tags 1 each.

        # ---- static tiles -------------------------------------------------
        w2t = wpool.tile([128, 1152], BF16, tag="w2")
        w3t = wpool.tile([128, 2304], BF16, tag="w3")
        w4t = wpool.tile([128, 4608], BF16, tag="w4")
        w1hit = wpool.tile([81, 128], BF16, tag="w1hi")
        w1lot = wpool.tile([54, 128], BF16, tag="w1lo")
        fc2t = wpool.tile([128, 80], BF16, tag="fc2")
        ntt = {}
        for i in range(1, 8):
            ntt[i] = wpool.tile([128, {1: 1, 2: 1, 3: 2, 4: 2, 5: 4, 6: 4, 7: 8}[i]],
                                F32, tag=f"nt{i}", name=f"ntt{i}")
        ident = wpool.tile([128, 128], F32, tag="ident")
        sc10 = wpool.tile([10, 1], F32, tag="sc10")

        # activations (replicate-padded, bf16, +-1)
        A2 = apool.tile([128, B * 34 * 34], BF16, tag="A2")
        A3 = apool.tile([128, B * 18 * 18], BF16, tag="A3")
        A4 = [apool.tile([128, B * 18 * 18], BF16, tag=f"A4_{i}", name=f"A4_{i}")
              for i in range(2)]
        A5 = [apool.tile([128, B * 10 * 10], BF16, tag=f"A5_{i}", name=f"A5_{i}")
              for i in range(2)]
        A6 = [apool.tile([128, B * 10 * 10], BF16, tag=f"A6_{i}", name=f"A6_{i}")
              for i in range(4)]
        F = [apool.tile([128, B * 16], BF16, tag=f"F_{i}", name=f"F_{i}")
             for i in range(4)]

        # ---- load weights / constants ------------------------------------
        nc.sync.dma_start(out=w1hit[:], in_=d["w1hi"][:])
        nc.sync.dma_start(out=w1lot[:], in_=d["w1lo"][:])
        for i, t in [(2, w2t), (3, w3t), (4, w4t)]:
            _dma_in(nc, t[:], d[f"w{i}"][:], splits=4)
        nc.sync.dma_start(out=fc2t[:], in_=d["fc2"][:].rearrange("p a b -> p (a b)"))
        for i in range(1, 8):
            nc.sync.dma_start(out=ntt[i][:], in_=d[f"nt{i}"][:])
        make_identity(nc, ident[:])
        nc.sync.dma_start(out=sc10[:], in_=d["scale_in"][:])

        # ---- conv1 (streamed in quarters of 4 images) ---------------------
        A2v = A2[:].rearrange("p (b y x) -> p b y x", b=B, y=34, x=34)
        with tc.tile_pool(name="c1pool", bufs=2) as c1pool, \
             tc.tile_pool(name="psum_c1", bufs=2, space="PSUM") as psum_c1:
            for q in range(4):
                hit = c1pool.tile([81, 4096], BF16, tag="hi", name="hit")
                lot = c1pool.tile([54, 4096], BF16, tag="lo", name="lot")
                _dma_in(nc, hit[:], d["hi"][:, q * 4096 : (q + 1) * 4096], splits=4)
                _dma_in(nc, lot[:], d["lo"][:, q * 4096 : (q + 1) * 4096], splits=4)
                for il in range(4):
                    img = q * 4 + il
                    for h in range(2):
                        blk = slice((il * 2 + h) * 512, (il * 2 + h + 1) * 512)
                        psA = psum_c1.tile([128, 512], F32, tag="c1h", name="psA")
                        psB = psum_c1.tile([128, 512], F32, tag="c1l", name="psB")
                        nc.tensor.matmul(psA[:], w1hit[:], hit[:, blk])
                        nc.tensor.matmul(psB[:], w1lot[:], lot[:, blk])
                        e = tmp.tile([128, 512], F32, tag="c1e", bufs=2)
                        e2 = tmp.tile([128, 512], F32, tag="c1e2", bufs=2)
                        # e = (S_lo + (-t)) + S_hi (validated fp32 order);
                        # two ops: only one PSUM input allowed per instruction
                        nc.vector.tensor_scalar_add(e[:], psB[:], ntt[1][:, 0:1])
                        nc.vector.tensor_add(e2[:], psA[:], e[:])
                        nc.scalar.activation(
                            A2v[:, img, 1 + h * 16 : 17 + h * 16, 1:33],
                            e2[:].rearrange("p (y x) -> p y x", y=16, x=32),
                            AF.Sign)

        # late weights reuse conv1's freed space
        wlate = est.enter_context(tc.tile_pool(name="wlate", bufs=1))
        w5t = wlate.tile([128, 9216], BF16, tag="w5")
        w6t = wlate.tile([128, 18432], BF16, tag="w6")
        _dma_in(nc, w5t[:], d["w5"][:], splits=4)
        _dma_in(nc, w6t[:], d["w6"][:], splits=8)

        _borders(nc, A2v, B, 34)

        # ---- conv layers --------------------------------------------------
        w3v = w3t[:].rearrange("p (kc mc t m) -> p kc mc t m", kc=1, mc=2, t=9, m=128)
        w2v = w2t[:].rearrange("p (kc mc t m) -> p kc mc t m", kc=1, mc=1, t=9, m=128)
        w4v = w4t[:].rearrange("p (kc mc t m) -> p kc mc t m", kc=2, mc=2, t=9, m=128)
        w5v = w5t[:].rearrange("p (kc mc t m) -> p kc mc t m", kc=2, mc=4, t=9, m=128)
        w6v = w6t[:].rearrange("p (kc mc t m) -> p kc mc t m", kc=4, mc=4, t=9, m=128)

        A3v = A3[:].rearrange("p (b y x) -> p b y x", b=B, y=18, x=18)
        A4v = [a[:].rearrange("p (b y x) -> p b y x", b=B, y=18, x=18) for a in A4]
        A5v = [a[:].rearrange("p (b y x) -> p b y x", b=B, y=10, x=10) for a in A5]
        A6v = [a[:].rearrange("p (b y x) -> p b y x", b=B, y=10, x=10) for a in A6]
        Fv = [f[:].rearrange("p (b y x) -> p b y x", b=B, y=4, x=4) for f in F]

        # conv2: 32x32 -> pool -> A3 interior
        for img in range(B):
            for h in range(2):
                ps = psum.tile([128, 512], F32, tag="cps")
                for tap in range(9):
                    dy, dx = tap // 3, tap % 3
                    nc.tensor.matmul(
                        ps[:], w2v[:, 0, 0, tap, :],
                        A2v[:, img, h * 16 + dy : h * 16 + dy + 16, dx : dx + 32],
                        start=(tap == 0), stop=(tap == 8))
                sg = tmp.tile([128, 512], BF16, tag="sg")
                nc.scalar.activation(sg[:], ps[:], AF.Sign, bias=ntt[2][:, 0:1])
                pm = tmp.tile([128, 256], BF16, tag="pm")
                sgv = sg[:].rearrange("p (y x two) -> p y x two", y=16, x=16, two=2)
                nc.vector.tensor_max(
                    pm[:].rearrange("p (y x) -> p y x", y=16, x=16),
                    sgv[:, :, :, 0], sgv[:, :, :, 1])
                pmv = pm[:].rearrange("p (yy two x) -> p yy two x", yy=8, two=2, x=16)
                nc.vector.tensor_max(
                    A3v[:, img, 1 + h * 8 : 9 + h * 8, 1:17],
                    pmv[:, :, 0, :], pmv[:, :, 1, :])
        _borders(nc, A3v, B, 18)

        # conv3: A3 -> A4 interior (no pool), blocks of 2 imgs
        for i in range(B // 2):
            for mc in range(2):
                ps = psum.tile([128, 512], F32, tag="cps")
                for tap in range(9):
                    dy, dx = tap // 3, tap % 3
                    nc.tensor.matmul(
                        ps[:], w3v[:, 0, mc, tap, :],
                        A3v[:, 2 * i : 2 * i + 2, dy : dy + 16, dx : dx + 16],
                        start=(tap == 0), stop=(tap == 8))
                nc.scalar.activation(
                    A4v[mc][:, 2 * i : 2 * i + 2, 1:17, 1:17],
                    ps[:].rearrange("p (b y x) -> p b y x", b=2, y=16, x=16),
                    AF.Sign, bias=ntt[3][:, mc : mc + 1])
        for mc in range(2):
            _borders(nc, A4v[mc], B, 18)

        # conv4: A4 -> pool -> A5 interior
        for i in range(B // 2):
            for mc in range(2):
                ps = psum.tile([128, 512], F32, tag="cps")
                k = 0
                for kc in range(2):
                    for tap in range(9):
                        dy, dx = tap // 3, tap % 3
                        nc.tensor.matmul(
                            ps[:], w4v[:, kc, mc, tap, :],
                            A4v[kc][:, 2 * i : 2 * i + 2, dy : dy + 16, dx : dx + 16],
                            start=(k == 0), stop=(k == 17))
                        k += 1
                sg = tmp.tile([128, 512], BF16, tag="sg")
                nc.scalar.activation(sg[:], ps[:], AF.Sign, bias=ntt[4][:, mc : mc + 1])
                pm = tmp.tile([128, 256], BF16, tag="pm")
                sgv = sg[:].rearrange("p (b y x two) -> p b y x two", b=2, y=16, x=8, two=2)
                nc.vector.tensor_max(
                    pm[:].rearrange("p (b y x) -> p b y x", b=2, y=16, x=8),
                    sgv[:, :, :, :, 0], sgv[:, :, :, :, 1])
                pmv = pm[:].rearrange("p (b yy two x) -> p b yy two x", b=2, yy=8, two=2, x=8)
                nc.vector.tensor_max(
                    A5v[mc][:, 2 * i : 2 * i + 2, 1:9, 1:9],
                    pmv[:, :, :, 0, :], pmv[:, :, :, 1, :])
        for mc in range(2):
            _borders(nc, A5v[mc], B, 10)

        # conv5: A5 -> A6 interior (no pool), blocks of 8 imgs
        for i in range(B // 8):
            for mc in range(4):
                ps = psum.tile([128, 512], F32, tag="cps")
                k = 0
                for kc in range(2):
                    for tap in range(9):
                        dy, dx = tap // 3, tap % 3
                        nc.tensor.matmul(
                            ps[:], w5v[:, kc, mc, tap, :],
                            A5v[kc][:, 8 * i : 8 * i + 8, dy : dy + 8, dx : dx + 8],
                            start=(k == 0), stop=(k == 17))
                        k += 1
                nc.scalar.activation(
                    A6v[mc][:, 8 * i : 8 * i + 8, 1:9, 1:9],
                    ps[:].rearrange("p (b y x) -> p b y x", b=8, y=8, x=8),
                    AF.Sign, bias=ntt[5][:, mc : mc + 1])
        for mc in range(4):
            _borders(nc, A6v[mc], B, 10)

        # conv6: A6 -> pool -> F
        for i in range(B // 8):
            for mc in range(4):
                ps = psum.tile([128, 512], F32, tag="cps")
                k = 0
                for kc in range(4):
                    for tap in range(9):
                        dy, dx = tap // 3, tap % 3
                        nc.tensor.matmul(
                            ps[:], w6v[:, kc, mc, tap, :],
                            A6v[kc][:, 8 * i : 8 * i + 8, dy : dy + 8, dx : dx + 8],
                            start=(k == 0), stop=(k == 35))
                        k += 1
                sg = tmp.tile([128, 512], BF16, tag="sg")
                nc.scalar.activation(sg[:], ps[:], AF.Sign, bias=ntt[6][:, mc : mc + 1])
                pm = tmp.tile([128, 256], BF16, tag="pm")
                sgv = sg[:].rearrange("p (b y x two) -> p b y x two", b=8, y=8, x=4, two=2)
                nc.vector.tensor_max(
                    pm[:].rearrange("p (b y x) -> p b y x", b=8, y=8, x=4),
                    sgv[:, :, :, :, 0], sgv[:, :, :, :, 1])
                pmv = pm[:].rearrange("p (b yy two x) -> p b yy two x", b=8, yy=4, two=2, x=4)
                nc.vector.tensor_max(
                    Fv[mc][:, 8 * i : 8 * i + 8, :, :],
                    pmv[:, :, :, 0, :], pmv[:, :, :, 1, :])

        # ---- fc1: psum[16, 512]x2 accumulated over 64 (cg,pix) steps ------
        psum_fc = est.enter_context(tc.tile_pool(name="psum_fc", bufs=1, space="PSUM"))
        with tc.tile_pool(name="fc1pool", bufs=12) as fc1pool:
            pa = psum_fc.tile([16, 512], F32, tag="fc1a")
            pb = psum_fc.tile([16, 512], F32, tag="fc1b")
            Fv2 = [f[:].rearrange("p (b pix) -> p b pix", b=B, pix=16) for f in F]
            for s in range(64):
                cg, pix = s // 16, s % 16
                wt = fc1pool.tile([128, 1024], BF16, tag="fc1w")
                nc.sync.dma_start(out=wt[:], in_=d["fc1"][s])
                lhs = Fv2[cg][:, :, pix]  # [128, 16] stride-16 cols
                nc.tensor.matmul(pa[:], lhs, wt[:, 0:512],
                                 start=(s == 0), stop=(s == 63))
                nc.tensor.matmul(pb[:], lhs, wt[:, 512:1024],
                                 start=(s == 0), stop=(s == 63))

        # ---- BN7 + sign + transpose + fc2 ---------------------------------
        e1 = tmp.tile([16, 1024], F32, tag="e1", bufs=1)
        nc.scalar.copy(e1[:, 0:512], pa[:])
        nc.scalar.copy(e1[:, 512:1024], pb[:])
        pT = psum_fc.tile([128, 128], F32, tag="pT")
        for j in range(8):
            nc.tensor.transpose(pT[:, 16 * j : 16 * j + 16],
                                e1[:, 128 * j : 128 * (j + 1)],
                                ident[0:16, 0:16])
        h1 = tmp.tile([128, 128], BF16, tag="h1", bufs=1)
        for j in range(8):
            nc.scalar.activation(h1[:, 16 * j : 16 * j + 16],
                                 pT[:, 16 * j : 16 * j + 16],
                                 mybir.ActivationFunctionType.Sign,
                                 bias=ntt[7][:, j : j + 1])
        fc2v = fc2t[:].rearrange("p (j m) -> p j m", j=8, m=10)
        p2 = psum_fc.tile([10, 16], F32, tag="p2")
        for j in range(8):
            nc.tensor.matmul(p2[:], fc2v[:, j, :], h1[:, 16 * j : 16 * j + 16],
                             start=(j == 0), stop=(j == 7))
        res = tmp.tile([10, 16], F32, tag="res", bufs=1)
        nc.scalar.activation(res[:], p2[:],
                             mybir.ActivationFunctionType.Copy, scale=sc10[:])
        # out[b, oc] <- res[oc, b]
        nc.sync.dma_start(
            out=d["out"][:].rearrange("b o -> o b"), in_=res[:])


def _borders(nc, Av, b, p):
    """Replicate-pad borders of padded layout [128, b, p, p] (interior filled)."""
    h = p - 2
    nc.vector.tensor_copy(Av[:, :, 1 : 1 + h, 0], Av[:, :, 1 : 1 + h, 1])
    nc.vector.tensor_copy(Av[:, :, 1 : 1 + h, p - 1], Av[:, :, 1 : 1 + h, p - 2])
    nc.vector.tensor_copy(Av[:, :, 0, :], Av[:, :, 1, :])
    nc.vector.tensor_copy(Av[:, :, p - 1, :], Av[:, :, p - 2, :])


# ---------------------------------------------------------------------------
# Entry point
# ---------------------------------------------------------------------------

_CACHED = {}


def _get_program():
    if "nc" not in _CACHED:
        _CACHED["nc"] = build_program()
    return _CACHED["nc"]


def kernel(**inputs) -> np.ndarray:
    from concourse import bass_utils

    in_maps = _prep(inputs)
    nc = _get_program()
    res = bass_utils.run_bass_kernel_spmd(nc, in_maps, list(range(N_CORES)))
    out = np.concatenate([np.asarray(r["out"], np.float32) for r in res.results], axis=0)
    return out
